# revision 31
# baseline (speedup 1.0000x reference)
"""Trainium2 Bass kernel for nn_Attention_GLM_Wrapped (S=2048, B=2, D=4096, H=32).

v2 fallback: baseline structure + startup/queue/prefetch scheduling fixes.
Measured 1,460,426 ns, rel err 1.395e-3.
"""
import sys

sys.path.insert(0, "/opt/trn_rl_repo")

import numpy as np
import ml_dtypes
from contextlib import ExitStack

import concourse.bass as bass
from concourse import bacc
import concourse.mybir as mybir
import concourse.tile as tile
from concourse.bass_utils import run_bass_kernel_spmd
from concourse.masks import make_identity

F32 = mybir.dt.float32
BF16 = mybir.dt.bfloat16
FP16 = mybir.dt.float16
AF = mybir.ActivationFunctionType

MMD = FP16
EXPB = -10.0

S, B, D = 2048, 2, 4096
H = 32
HD = 128
T = S * B
NC = 8
HPC = H // NC
EH = HPC * HD
TPC = T // NC // B
SCALE = float(1.0 / np.sqrt(HD))

_cache = {}


def _np_mmd(a):
    if MMD == BF16:
        return np.asarray(a, np.float32).astype(ml_dtypes.bfloat16)
    if MMD == FP16:
        return np.asarray(a, np.float32).astype(np.float16)
    return np.ascontiguousarray(np.asarray(a, np.float32))


def _rope_tables():
    rot = 64
    inv_freq = 1.0 / (10000.0 ** (np.arange(0, rot, 2, dtype=np.float32) / rot))
    v = np.arange(S, dtype=np.float32)[:, None] * inv_freq[None, :]
    v = np.concatenate([v, v], axis=-1)
    return np.cos(v).astype(np.float32), np.sin(v).astype(np.float32)


def build_program():
    nc = bacc.Bacc("TRN2", target_bir_lowering=False, debug=False, num_devices=NC)

    xS = nc.dram_tensor("xS", [HD, T // HD, D // HD, HD], MMD,
                        kind="ExternalInput").ap()
    wqT = nc.dram_tensor("wqT", [HD, D // HD, EH], MMD, kind="ExternalInput").ap()
    wkT = nc.dram_tensor("wkT", [HD, D // HD, EH], MMD, kind="ExternalInput").ap()
    wvT = nc.dram_tensor("wvT", [HD, D // HD, EH], MMD, kind="ExternalInput").ap()
    woS = nc.dram_tensor("woS", [D // 512, HD, D // HD, 512], MMD,
                         kind="ExternalInput").ap()
    bq = nc.dram_tensor("bq", [HD, EH], F32, kind="ExternalInput").ap()
    bk = nc.dram_tensor("bk", [HD, EH], F32, kind="ExternalInput").ap()
    bv = nc.dram_tensor("bv", [HD, EH], F32, kind="ExternalInput").ap()
    bo = nc.dram_tensor("bo", [HD, D], F32, kind="ExternalInput").ap()
    cosN = nc.dram_tensor("cosN", [T, HD], MMD, kind="ExternalInput").ap()
    sinN = nc.dram_tensor("sinN", [T, HD], MMD, kind="ExternalInput").ap()
    onesc = nc.dram_tensor("onesc", [HD, HD], MMD, kind="ExternalInput").ap()
    out = nc.dram_tensor("out", [B, TPC, D], F32, kind="ExternalOutput").ap()

    NTB = T // HD
    NDC = D // HD

    with tile.TileContext(nc) as tc, ExitStack() as top:
        dram = top.enter_context(tc.tile_pool(name="dram", bufs=1, space="DRAM"))
        cpool = top.enter_context(tc.tile_pool(name="cpool", bufs=1))

        # per-batch spill tiles: batch-0 attention loads depend only on the
        # batch-0 half of phase A, so its q/k/v can stream in early
        qT_db = [dram.tile([EH, S], MMD, name=f"qT_d{b}") for b in range(B)]
        kT_db = [dram.tile([EH, S], MMD, name=f"kT_d{b}") for b in range(B)]
        v_db = [dram.tile([HPC, HD, S // HD, HD], MMD, name=f"v_d{b}")
                for b in range(B)]
        cc_in = [dram.tile([NC, EH, TPC], MMD, name=f"cc_in_{b}")
                 for b in range(B)]
        cc_out = [dram.tile([NC, EH, TPC], MMD, name=f"cc_out_{b}")
                  for b in range(B)]

        ident = cpool.tile([HD, HD], MMD)
        make_identity(nc, ident)

        with ExitStack() as wctx:
            pw = wctx.enter_context(tc.tile_pool(name="pw", bufs=1, space="PSUM"))
            pwr = cpool.tile([HD, 512], MMD)
            nc.vector.memset(pwr[:], 0.0)
            pwt = pw.tile([HD, 512], F32)
            NPW = 125
            for i in range(NPW):
                nc.tensor.matmul(pwt[:], ident[:], pwr[:],
                                 start=(i == 0), stop=(i == NPW - 1))

        ones_sb = cpool.tile([HD, HD], MMD)
        nc.gpsimd.dma_start(ones_sb[:], onesc[:])
        bq_sb = cpool.tile([HD, EH], F32)
        nc.gpsimd.dma_start(bq_sb[:], bq[:])
        bk_sb = cpool.tile([HD, EH], F32)
        nc.gpsimd.dma_start(bk_sb[:], bk[:])
        bv_sb = cpool.tile([HD, EH], F32)
        nc.gpsimd.dma_start(bv_sb[:], bv[:])
        expb_sb = cpool.tile([HD, 1], F32)
        nc.vector.memset(expb_sb[:], EXPB)

        # attention input pool lives at top level so batch-0 head loads can
        # be issued from inside phase A (overlapping the batch-1 blocks)
        NKC = S // HD
        qk = top.enter_context(tc.tile_pool(name="qk", bufs=2))
        qkv_tiles = {}

        def load_qkv(b, hl):
            esl = slice(hl * HD, (hl + 1) * HD)
            qh = qk.tile([HD, S], MMD, tag="qh", name=f"qh_{b}_{hl}")
            nc.gpsimd.dma_start(qh[:], qT_db[b][esl, :])
            kh = qk.tile([HD, S], MMD, tag="kh", name=f"kh_{b}_{hl}")
            nc.gpsimd.dma_start(kh[:], kT_db[b][esl, :])
            vh = qk.tile([HD, NKC, HD], MMD, tag="vh", name=f"vh_{b}_{hl}")
            nc.gpsimd.dma_start(vh[:], v_db[b][hl])
            qkv_tiles[(b, hl)] = (qh, kh, vh)

        with ExitStack() as ctx:
            wres = ctx.enter_context(tc.tile_pool(name="wres", bufs=1))
            xp = ctx.enter_context(tc.tile_pool(name="xp", bufs=3))
            rp = ctx.enter_context(tc.tile_pool(name="rp", bufs=3))
            op = ctx.enter_context(tc.tile_pool(name="op", bufs=6))
            ps = ctx.enter_context(tc.tile_pool(name="psA", bufs=4, space="PSUM"))
            pst = ctx.enter_context(tc.tile_pool(name="psAt", bufs=4, space="PSUM"))

            wqS = wres.tile([HD, NDC, EH], MMD)
            wkS = wres.tile([HD, NDC, EH], MMD)
            wvS = wres.tile([HD, NDC, EH], MMD)
            xo0 = xp.tile([HD, NDC, HD], MMD, tag="xo", name="xo_0")
            nc.scalar.dma_start(xo0[:], xS[:, 0])
            cos0 = xp.tile([HD, HD], MMD, tag="cos", name="cos_0")
            nc.gpsimd.dma_start(cos0[:], cosN[0:HD, :])
            sin0 = xp.tile([HD, HD], MMD, tag="sin", name="sin_0")
            nc.gpsimd.dma_start(sin0[:], sinN[0:HD, :])
            for ch in range(8):
                csl = slice(ch * NDC // 8, (ch + 1) * NDC // 8)
                for wS, wsrc, q in ((wqS, wqT, nc.sync), (wkS, wkT, nc.scalar),
                                    (wvS, wvT, nc.gpsimd)):
                    q.dma_start(wS[:, csl], wsrc[:, csl])

            for tb in range(NTB):
                tsl = slice(tb * HD, (tb + 1) * HD)
                if tb == 0:
                    xo, cos_t, sin_t = xo0, cos0, sin0
                else:
                    xo = xp.tile([HD, NDC, HD], MMD, tag="xo")
                    nc.scalar.dma_start(xo[:], xS[:, tb])
                    cos_t = xp.tile([HD, HD], MMD, tag="cos")
                    nc.scalar.dma_start(cos_t[:], cosN[tsl, :])
                    sin_t = xp.tile([HD, HD], MMD, tag="sin")
                    nc.scalar.dma_start(sin_t[:], sinN[tsl, :])

                b_ = tb // (NTB // B)
                tlsl = slice((tb % (NTB // B)) * HD,
                             (tb % (NTB // B) + 1) * HD)
                for name, wS, b_sb in (("q", wqS, bq_sb), ("k", wkS, bk_sb)):
                    outd = qT_db[b_] if name == "q" else kT_db[b_]
                    pq = ps.tile([HD, EH], F32, tag="pqk", name=f"pqk_{name}_{tb}")
                    for d in range(NDC):
                        nc.tensor.matmul(pq[:], xo[:, d], wS[:, d],
                                         start=(d == 0), stop=(d == NDC - 1))
                    qb = rp.tile([HD, EH], MMD, tag="qb", name=f"qb_{name}_{tb}")
                    nc.vector.tensor_tensor(
                        qb[:], pq[:], b_sb[:], mybir.AluOpType.add)
                    rq = rp.tile([HD, EH], MMD, tag="rq", name=f"rq_{name}_{tb}")
                    qb4 = qb.rearrange("p (h e) -> p h e", h=HPC)
                    rq4 = rq.rearrange("p (h e) -> p h e", h=HPC)
                    cosb = cos_t[:, None, :].to_broadcast([HD, HPC, HD])
                    nc.vector.tensor_tensor(rq4[:], qb4[:], cosb,
                                            mybir.AluOpType.mult)
                    qb8 = qb.rearrange("p (h u e) -> p h u e", h=HPC, u=4)
                    rq8 = rq.rearrange("p (h u e) -> p h u e", h=HPC, u=4)
                    sin8 = sin_t.rearrange("p (u e) -> p u e", u=4)
                    tmp = rp.tile([HD, HPC, 2, 32], MMD, tag="tmp",
                                  name=f"tmp_{name}_{tb}")
                    nc.vector.tensor_tensor(
                        tmp[:], qb8[:, :, 1::2, :],
                        sin8[:, None, 0::2, :].to_broadcast([HD, HPC, 2, 32]),
                        mybir.AluOpType.mult)
                    nc.vector.tensor_tensor(
                        rq8[:, :, 0::2, :], rq8[:, :, 0::2, :], tmp[:],
                        mybir.AluOpType.add)
                    nc.vector.tensor_tensor(
                        tmp[:], qb8[:, :, 0::2, :],
                        sin8[:, None, 1::2, :].to_broadcast([HD, HPC, 2, 32]),
                        mybir.AluOpType.mult)
                    nc.vector.tensor_tensor(
                        rq8[:, :, 1::2, :], rq8[:, :, 1::2, :], tmp[:],
                        mybir.AluOpType.add)
                    for hl in range(HPC):
                        ptr = pst.tile([HD, HD], MMD, tag="ptr",
                                       name=f"ptr_{name}_{tb}_{hl}")
                        nc.tensor.transpose(ptr[:], rq[:, hl * HD:(hl + 1) * HD],
                                            ident[:])
                        ob = op.tile([HD, HD], MMD, tag="ob",
                                     name=f"ob_{name}_{tb}_{hl}")
                        nc.scalar.copy(ob[:], ptr[:])
                        nc.sync.dma_start(outd[hl * HD:(hl + 1) * HD, tlsl],
                                          ob[:])

                pv = ps.tile([HD, EH], F32, tag="pqk", name=f"pv_{tb}")
                for d in range(NDC):
                    nc.tensor.matmul(pv[:], xo[:, d], wvS[:, d],
                                     start=(d == 0), stop=(d == NDC - 1))
                vb = op.tile([HD, EH], MMD, tag="vb", name=f"vb_{tb}")
                nc.vector.tensor_tensor(
                    vb[:], pv[:], bv_sb[:], mybir.AluOpType.add)
                for hl in range(HPC):
                    nc.sync.dma_start(v_db[b_][hl, :, tb % (NTB // B), :],
                                      vb[:, hl * HD:(hl + 1) * HD])
                if tb == NTB // B - 1:
                    # batch-0 spills complete: stream its first attention
                    # heads into SBUF while the batch-1 blocks project
                    load_qkv(0, 0)
                    load_qkv(0, 1)

        NPR = NKC // 2
        NQT = S // 512
        NES = D // 512
        with ExitStack() as ctx:
            pp = ctx.enter_context(tc.tile_pool(name="pp", bufs=5))
            accp = ctx.enter_context(tc.tile_pool(name="accp", bufs=2))
            ao = ctx.enter_context(tc.tile_pool(name="ao", bufs=4))
            wvf = ctx.enter_context(tc.tile_pool(name="wvf", bufs=1))
            wop = ctx.enter_context(tc.tile_pool(name="wop", bufs=6))
            oo = ctx.enter_context(tc.tile_pool(name="oo", bufs=4))
            psl = ctx.enter_context(tc.tile_pool(name="psl", bufs=3, space="PSUM"))
            pso = ctx.enter_context(tc.tile_pool(name="pso", bufs=2, space="PSUM"))

            bo_sb = wvf.tile([HD, D], F32)
            nc.sync.dma_start(bo_sb[:], bo[:])
            wvfS = {}
            wo_cache = {}

            def fire_a2a(b):
                nc.gpsimd.collective_compute(
                    "AllToAll", mybir.AluOpType.bypass,
                    replica_groups=[list(range(NC))],
                    ins=[cc_in[b][:]], outs=[cc_out[b][:]],
                )

            def stage_wvf(b):
                wvfS[b] = wvf.tile([HD, NDC, TPC], MMD, name=f"wvfS_{b}")
                t_ = wvfS[b]
                for i in range(NC):
                    nc.gpsimd.dma_start(
                        t_[:, i * HPC:(i + 1) * HPC, :],
                        cc_out[b][i].rearrange("(r1 p) c -> p r1 c", p=HD))

            def prefetch_wo(es):
                if es in wo_cache:
                    return
                wlo = wop.tile([HD, NDC // 2, 512], MMD, tag="wo")
                nc.sync.dma_start(wlo[:], woS[es, :, 0:NDC // 2])
                whi = wop.tile([HD, NDC // 2, 512], MMD, tag="wo")
                nc.scalar.dma_start(whi[:], woS[es, :, NDC // 2:NDC])
                wo_cache[es] = (wlo, whi)
                while len(wo_cache) > 3:
                    del wo_cache[next(iter(wo_cache))]

            def emit_d_unit(b, es, tb2):
                esl = slice(es * 512, (es + 1) * 512)
                prefetch_wo(es)
                wo_lo, wo_hi = wo_cache[es]
                pd = pso.tile([HD, 512], tag="po", dtype=F32,
                              name=f"pd_{b}_{es}_{tb2}")
                for di, d in enumerate(range(NDC)):
                    wo_t = wo_lo if d < NDC // 2 else wo_hi
                    nc.tensor.matmul(
                        pd[:],
                        wvfS[b][:, d, tb2 * HD:(tb2 + 1) * HD],
                        wo_t[:, d % (NDC // 2)],
                        start=(di == 0), stop=(di == NDC - 1))
                ob = oo.tile([HD, 512], F32, tag="obD",
                             name=f"obD_{b}_{es}_{tb2}")
                nc.vector.tensor_tensor(
                    ob[:], pd[:], bo_sb[:, esl], mybir.AluOpType.add)
                nc.scalar.dma_start(
                    out[b, tb2 * HD:(tb2 + 1) * HD, esl], ob[:])

            d_queue = []

            def drain_d(n=1):
                for _ in range(n):
                    if not d_queue:
                        return
                    b_, es_, tb2_ = d_queue.pop(0)
                    if d_queue:
                        prefetch_wo(d_queue[min(1, len(d_queue) - 1)][1])
                    emit_d_unit(b_, es_, tb2_)

            def emit_attention(b):
                # flat unit list across heads: the consume/finish pipeline
                # never flushes at head boundaries
                units = [(hl, qt, j) for hl in range(HPC)
                         for qt in range(NQT) for j in range(NPR)]
                po = {}
                acc = {}
                pls = {}
                tiles = {}

                def consume(hl, qt, j):
                    vh = tiles[hl][2]
                    pl = pls.pop((hl, qt, j))
                    pe2 = pp.tile([HD, 1024], MMD, tag="pe",
                                  name=f"pe_{b}_{hl}_{qt}_{j}")
                    nc.scalar.activation(pe2[:], pl[:], AF.Exp,
                                         scale=SCALE, bias=expb_sb[:])
                    if j == 0:
                        nc.vector.tensor_tensor(
                            acc[(hl, qt)][:], pe2[:, 0:512], pe2[:, 512:1024],
                            mybir.AluOpType.add)
                    else:
                        nc.vector.tensor_tensor(
                            acc[(hl, qt)][:], acc[(hl, qt)][:], pe2[:, 0:512],
                            mybir.AluOpType.add)
                        nc.vector.tensor_tensor(
                            acc[(hl, qt)][:], acc[(hl, qt)][:],
                            pe2[:, 512:1024], mybir.AluOpType.add)
                    nc.tensor.matmul(po[(hl, qt)][:], vh[:, 2 * j],
                                     pe2[:, 0:512],
                                     start=(j == 0), stop=False)
                    nc.tensor.matmul(po[(hl, qt)][:], vh[:, 2 * j + 1],
                                     pe2[:, 512:1024],
                                     start=False, stop=(j == NPR - 1))

                def finish_qt(hl, qt):
                    esl = slice(hl * HD, (hl + 1) * HD)
                    su = psl.tile([HD, 1024], F32, tag="pl",
                                  name=f"su_{b}_{hl}_{qt}")
                    nc.tensor.matmul(su[:, 0:512], ones_sb[:],
                                     acc[(hl, qt)][:], start=True, stop=True)
                    rec = ao.tile([HD, 512], F32, tag="rec",
                                  name=f"rec_{b}_{hl}_{qt}")
                    nc.vector.reciprocal_approx_fast(rec[:], su[:, 0:512])
                    osb = ao.tile([HD, 512], MMD, tag="osb",
                                  name=f"osb_{b}_{hl}_{qt}")
                    nc.vector.tensor_tensor(
                        osb[:], po[(hl, qt)][:], rec[:], mybir.AluOpType.mult)
                    del po[(hl, qt)], acc[(hl, qt)]
                    for j2 in range(2):
                        j = qt * 2 + j2
                        nc.sync.dma_start(
                            cc_in[b][j, esl, :],
                            osb[:, j2 * TPC:(j2 + 1) * TPC])

                def group_before(hl, qt):
                    return (hl, qt - 1) if qt > 0 else (hl - 1, NQT - 1)

                for idx, (hl, qt, j) in enumerate(units):
                    if qt == 0 and j == 0:
                        if (b, hl) not in qkv_tiles:
                            load_qkv(b, hl)
                        tiles[hl] = qkv_tiles.pop((b, hl))
                        tiles.pop(hl - 2, None)
                    qh, kh, vh = tiles[hl]
                    if j == 0:
                        po[(hl, qt)] = pso.tile([HD, 512], tag="po", dtype=F32,
                                                name=f"po_{b}_{hl}_{qt}")
                        acc[(hl, qt)] = accp.tile([HD, 512], MMD, tag="acc",
                                                  name=f"acc_{b}_{hl}_{qt}")
                    qsl = slice(qt * 512, (qt + 1) * 512)
                    pl = psl.tile([HD, 1024], F32, tag="pl",
                                  name=f"pl_{b}_{hl}_{qt}_{j}")
                    nc.tensor.matmul(
                        pl[:, 0:512],
                        kh[:, (2 * j) * HD:(2 * j + 1) * HD], qh[:, qsl],
                        start=True, stop=True)
                    nc.tensor.matmul(
                        pl[:, 512:1024],
                        kh[:, (2 * j + 1) * HD:(2 * j + 2) * HD], qh[:, qsl],
                        start=True, stop=True)
                    pls[(hl, qt, j)] = pl
                    if qt == 1 and j == 0 and not (b == B - 1 and hl == HPC - 1):
                        nb, nhl = (b, hl + 1) if hl + 1 < HPC else (b + 1, 0)
                        if (nb, nhl) not in qkv_tiles:
                            load_qkv(nb, nhl)
                    if idx >= 2:
                        consume(*units[idx - 2])
                    if j == 3 and idx >= 11:
                        fhl, fqt = group_before(hl, qt)
                        finish_qt(fhl, fqt)
                        if b == 1:
                            if fhl == 1 and fqt == 0 and d_queue:
                                stage_wvf(0)
                                prefetch_wo(d_queue[0][1])
                            if (fhl >= 3 or (fhl == 2 and fqt >= 2)) \
                                    and d_queue:
                                drain_d(1)
                consume(*units[-2])
                consume(*units[-1])
                finish_qt(HPC - 1, NQT - 1)
                if b == 1 and d_queue:
                    drain_d(2)
                fire_a2a(b)

            emit_attention(0)
            d_queue.extend((0, es, tb2)
                           for es in range(NES) for tb2 in range(TPC // HD))
            emit_attention(1)
            stage_wvf(1)
            d_queue.extend((1, es, tb2)
                           for es in reversed(range(NES))
                           for tb2 in range(TPC // HD))
            while d_queue:
                drain_d(1)

    nc.compile()
    return nc


def host_prep(x, position_ids, qkv_weight, qkv_bias, attn_out_weight,
              attn_out_bias):
    pos = np.asarray(position_ids).astype(np.int64)
    x = np.asarray(x, dtype=np.float32)
    Wqkv = np.asarray(qkv_weight, dtype=np.float32)
    bqkv = np.asarray(qkv_bias, dtype=np.float32)
    Wo = np.asarray(attn_out_weight, dtype=np.float32)
    bo = np.asarray(attn_out_bias, dtype=np.float32)

    xT = x.transpose(2, 1, 0).reshape(D, T)
    xS = _np_mmd(np.ascontiguousarray(
        xT.reshape(D // HD, HD, T // HD, HD).transpose(1, 2, 0, 3)))
    woS = _np_mmd(np.ascontiguousarray(
        Wo.T.reshape(D // HD, HD, D // 512, 512).transpose(2, 1, 0, 3)))

    cos_t, sin_t = _rope_tables()
    cosN = np.empty((T, HD), np.float32)
    sinN = np.empty((T, HD), np.float32)
    for b in range(B):
        rows = slice(b * S, (b + 1) * S)
        p1 = pos[b, 0, :]
        p2 = pos[b, 1, :]
        cosN[rows, 0:64] = cos_t[p1]
        cosN[rows, 64:128] = cos_t[p2]
        s1 = sin_t[p1].copy()
        s1[:, 0:32] *= -1.0
        s2 = sin_t[p2].copy()
        s2[:, 0:32] *= -1.0
        sinN[rows, 0:64] = s1
        sinN[rows, 64:128] = s2

    ones = _np_mmd(np.ones((HD, HD), np.float32))
    shared = dict(xS=xS, woS=woS, cosN=_np_mmd(cosN), sinN=_np_mmd(sinN),
                  onesc=ones,
                  bo=np.ascontiguousarray(np.broadcast_to(bo, (HD, D))))

    in_maps = []
    for c in range(NC):
        heads = range(HPC * c, HPC * (c + 1))
        wq = np.concatenate([Wqkv[384 * h: 384 * h + 128] for h in heads])
        wk = np.concatenate([Wqkv[384 * h + 128: 384 * h + 256] for h in heads])
        wv = np.concatenate([Wqkv[384 * h + 256: 384 * h + 384] for h in heads])
        in_maps.append(dict(
            shared,
            wqT=_np_mmd(np.ascontiguousarray(
                wq.T.reshape(D // HD, HD, EH).transpose(1, 0, 2))),
            wkT=_np_mmd(np.ascontiguousarray(
                wk.T.reshape(D // HD, HD, EH).transpose(1, 0, 2))),
            wvT=_np_mmd(np.ascontiguousarray(
                wv.T.reshape(D // HD, HD, EH).transpose(1, 0, 2))),
            bq=np.ascontiguousarray(np.broadcast_to(np.concatenate(
                [bqkv[384 * h: 384 * h + 128] for h in heads]), (HD, EH))),
            bk=np.ascontiguousarray(np.broadcast_to(np.concatenate(
                [bqkv[384 * h + 128: 384 * h + 256] for h in heads]), (HD, EH))),
            bv=np.ascontiguousarray(np.broadcast_to(np.concatenate(
                [bqkv[384 * h + 256: 384 * h + 384] for h in heads]), (HD, EH))),
        ))
    return in_maps


def kernel(x, position_ids, qkv_weight, qkv_bias, attn_out_weight,
           attn_out_bias, _trace=False):
    if "nc" not in _cache:
        _cache["nc"] = build_program()
    nc = _cache["nc"]

    in_maps = host_prep(x, position_ids, qkv_weight, qkv_bias,
                        attn_out_weight, attn_out_bias)
    res = run_bass_kernel_spmd(nc, in_maps, core_ids=list(range(NC)),
                               trace=_trace)
    _cache["last_result"] = res

    out = np.empty((S, B, D), np.float32)
    for c in range(NC):
        oc = res.results[c]["out"]
        for b in range(B):
            out[TPC * c: TPC * (c + 1), b, :] = oc[b]
    return out


# revision 32
# speedup vs baseline: 1.0066x; 1.0066x over previous
"""Trainium2 Bass kernel for nn_Attention_GLM_Wrapped (S=2048, B=2, D=4096, H=32).

v2 fallback: baseline structure + startup/queue/prefetch scheduling fixes.
Measured 1,460,426 ns, rel err 1.395e-3.
"""
import sys

sys.path.insert(0, "/opt/trn_rl_repo")

import numpy as np
import ml_dtypes
from contextlib import ExitStack

import concourse.bass as bass
from concourse import bacc
import concourse.mybir as mybir
import concourse.tile as tile
from concourse.bass_utils import run_bass_kernel_spmd
from concourse.masks import make_identity

F32 = mybir.dt.float32
BF16 = mybir.dt.bfloat16
FP16 = mybir.dt.float16
AF = mybir.ActivationFunctionType

MMD = FP16
EXPB = -10.0

S, B, D = 2048, 2, 4096
H = 32
HD = 128
T = S * B
NC = 8
HPC = H // NC
EH = HPC * HD
TPC = T // NC // B
SCALE = float(1.0 / np.sqrt(HD))

_cache = {}


def _np_mmd(a):
    if MMD == BF16:
        return np.asarray(a, np.float32).astype(ml_dtypes.bfloat16)
    if MMD == FP16:
        return np.asarray(a, np.float32).astype(np.float16)
    return np.ascontiguousarray(np.asarray(a, np.float32))


def _rope_tables():
    rot = 64
    inv_freq = 1.0 / (10000.0 ** (np.arange(0, rot, 2, dtype=np.float32) / rot))
    v = np.arange(S, dtype=np.float32)[:, None] * inv_freq[None, :]
    v = np.concatenate([v, v], axis=-1)
    return np.cos(v).astype(np.float32), np.sin(v).astype(np.float32)


def build_program():
    nc = bacc.Bacc("TRN2", target_bir_lowering=False, debug=False, num_devices=NC)

    xS = nc.dram_tensor("xS", [HD, T // HD, D // HD, HD], MMD,
                        kind="ExternalInput").ap()
    wqT = nc.dram_tensor("wqT", [HD, D // HD, EH], MMD, kind="ExternalInput").ap()
    wkT = nc.dram_tensor("wkT", [HD, D // HD, EH], MMD, kind="ExternalInput").ap()
    wvT = nc.dram_tensor("wvT", [HD, D // HD, EH], MMD, kind="ExternalInput").ap()
    woS = nc.dram_tensor("woS", [D // 512, HD, D // HD, 512], MMD,
                         kind="ExternalInput").ap()
    bq = nc.dram_tensor("bq", [HD, EH], F32, kind="ExternalInput").ap()
    bk = nc.dram_tensor("bk", [HD, EH], F32, kind="ExternalInput").ap()
    bv = nc.dram_tensor("bv", [HD, EH], F32, kind="ExternalInput").ap()
    bo = nc.dram_tensor("bo", [HD, D], F32, kind="ExternalInput").ap()
    cosN = nc.dram_tensor("cosN", [T, HD], MMD, kind="ExternalInput").ap()
    sinN = nc.dram_tensor("sinN", [T, HD], MMD, kind="ExternalInput").ap()
    onesc = nc.dram_tensor("onesc", [HD, HD], MMD, kind="ExternalInput").ap()
    out = nc.dram_tensor("out", [B, TPC, D], F32, kind="ExternalOutput").ap()

    NTB = T // HD
    NDC = D // HD

    with tile.TileContext(nc) as tc, ExitStack() as top:
        dram = top.enter_context(tc.tile_pool(name="dram", bufs=1, space="DRAM"))
        cpool = top.enter_context(tc.tile_pool(name="cpool", bufs=1))

        # per-batch spill tiles: batch-0 attention loads depend only on the
        # batch-0 half of phase A, so its q/k/v can stream in early
        qT_db = [dram.tile([EH, S], MMD, name=f"qT_d{b}") for b in range(B)]
        kT_db = [dram.tile([EH, S], MMD, name=f"kT_d{b}") for b in range(B)]
        v_db = [dram.tile([HPC, HD, S // HD, HD], MMD, name=f"v_d{b}")
                for b in range(B)]
        cc_in = [dram.tile([NC, EH, TPC], MMD, name=f"cc_in_{b}")
                 for b in range(B)]
        cc_out = [dram.tile([NC, EH, TPC], MMD, name=f"cc_out_{b}")
                  for b in range(B)]

        ident = cpool.tile([HD, HD], MMD)
        make_identity(nc, ident)

        with ExitStack() as wctx:
            pw = wctx.enter_context(tc.tile_pool(name="pw", bufs=1, space="PSUM"))
            pwr = cpool.tile([HD, 512], MMD)
            nc.vector.memset(pwr[:], 0.0)
            pwt = pw.tile([HD, 512], F32)
            NPW = 125
            for i in range(NPW):
                nc.tensor.matmul(pwt[:], ident[:], pwr[:],
                                 start=(i == 0), stop=(i == NPW - 1))

        ones_sb = cpool.tile([HD, HD], MMD)
        nc.gpsimd.dma_start(ones_sb[:], onesc[:])
        bq_sb = cpool.tile([HD, EH], F32)
        nc.gpsimd.dma_start(bq_sb[:], bq[:])
        bk_sb = cpool.tile([HD, EH], F32)
        nc.gpsimd.dma_start(bk_sb[:], bk[:])
        bv_sb = cpool.tile([HD, EH], F32)
        nc.gpsimd.dma_start(bv_sb[:], bv[:])
        expb_sb = cpool.tile([HD, 1], F32)
        nc.vector.memset(expb_sb[:], EXPB)

        # attention input pool lives at top level so batch-0 head loads can
        # be issued from inside phase A (overlapping the batch-1 blocks)
        NKC = S // HD
        qk = top.enter_context(tc.tile_pool(name="qk", bufs=2))
        qkv_tiles = {}

        def load_qkv(b, hl):
            esl = slice(hl * HD, (hl + 1) * HD)
            qh = qk.tile([HD, S], MMD, tag="qh", name=f"qh_{b}_{hl}")
            nc.gpsimd.dma_start(qh[:], qT_db[b][esl, :])
            kh = qk.tile([HD, S], MMD, tag="kh", name=f"kh_{b}_{hl}")
            nc.gpsimd.dma_start(kh[:], kT_db[b][esl, :])
            vh = qk.tile([HD, NKC, HD], MMD, tag="vh", name=f"vh_{b}_{hl}")
            nc.gpsimd.dma_start(vh[:], v_db[b][hl])
            qkv_tiles[(b, hl)] = (qh, kh, vh)

        with ExitStack() as ctx:
            wres = ctx.enter_context(tc.tile_pool(name="wres", bufs=1))
            xp = ctx.enter_context(tc.tile_pool(name="xp", bufs=3))
            rp = ctx.enter_context(tc.tile_pool(name="rp", bufs=3))
            op = ctx.enter_context(tc.tile_pool(name="op", bufs=6))
            ps = ctx.enter_context(tc.tile_pool(name="psA", bufs=4, space="PSUM"))
            pst = ctx.enter_context(tc.tile_pool(name="psAt", bufs=4, space="PSUM"))

            wqS = wres.tile([HD, NDC, EH], MMD)
            wkS = wres.tile([HD, NDC, EH], MMD)
            wvS = wres.tile([HD, NDC, EH], MMD)
            xo0 = xp.tile([HD, NDC, HD], MMD, tag="xo", name="xo_0")
            nc.scalar.dma_start(xo0[:], xS[:, 0])
            cos0 = xp.tile([HD, HD], MMD, tag="cos", name="cos_0")
            nc.gpsimd.dma_start(cos0[:], cosN[0:HD, :])
            sin0 = xp.tile([HD, HD], MMD, tag="sin", name="sin_0")
            nc.gpsimd.dma_start(sin0[:], sinN[0:HD, :])
            for ch in range(8):
                csl = slice(ch * NDC // 8, (ch + 1) * NDC // 8)
                for wS, wsrc, q in ((wqS, wqT, nc.sync), (wkS, wkT, nc.scalar),
                                    (wvS, wvT, nc.gpsimd)):
                    q.dma_start(wS[:, csl], wsrc[:, csl])

            for tb in range(NTB):
                tsl = slice(tb * HD, (tb + 1) * HD)
                if tb == 0:
                    xo, cos_t, sin_t = xo0, cos0, sin0
                else:
                    xo = xp.tile([HD, NDC, HD], MMD, tag="xo")
                    nc.scalar.dma_start(xo[:], xS[:, tb])
                    cos_t = xp.tile([HD, HD], MMD, tag="cos")
                    nc.scalar.dma_start(cos_t[:], cosN[tsl, :])
                    sin_t = xp.tile([HD, HD], MMD, tag="sin")
                    nc.scalar.dma_start(sin_t[:], sinN[tsl, :])

                b_ = tb // (NTB // B)
                tlsl = slice((tb % (NTB // B)) * HD,
                             (tb % (NTB // B) + 1) * HD)
                for name, wS, b_sb in (("q", wqS, bq_sb), ("k", wkS, bk_sb)):
                    outd = qT_db[b_] if name == "q" else kT_db[b_]
                    pq = ps.tile([HD, EH], F32, tag="pqk", name=f"pqk_{name}_{tb}")
                    for d in range(NDC):
                        nc.tensor.matmul(pq[:], xo[:, d], wS[:, d],
                                         start=(d == 0), stop=(d == NDC - 1))
                    qb = rp.tile([HD, EH], MMD, tag="qb", name=f"qb_{name}_{tb}")
                    nc.vector.tensor_tensor(
                        qb[:], pq[:], b_sb[:], mybir.AluOpType.add)
                    rq = rp.tile([HD, EH], MMD, tag="rq", name=f"rq_{name}_{tb}")
                    qb4 = qb.rearrange("p (h e) -> p h e", h=HPC)
                    rq4 = rq.rearrange("p (h e) -> p h e", h=HPC)
                    cosb = cos_t[:, None, :].to_broadcast([HD, HPC, HD])
                    nc.vector.tensor_tensor(rq4[:], qb4[:], cosb,
                                            mybir.AluOpType.mult)
                    qb8 = qb.rearrange("p (h u e) -> p h u e", h=HPC, u=4)
                    rq8 = rq.rearrange("p (h u e) -> p h u e", h=HPC, u=4)
                    sin8 = sin_t.rearrange("p (u e) -> p u e", u=4)
                    tmp = rp.tile([HD, HPC, 2, 32], MMD, tag="tmp",
                                  name=f"tmp_{name}_{tb}")
                    nc.vector.tensor_tensor(
                        tmp[:], qb8[:, :, 1::2, :],
                        sin8[:, None, 0::2, :].to_broadcast([HD, HPC, 2, 32]),
                        mybir.AluOpType.mult)
                    nc.vector.tensor_tensor(
                        rq8[:, :, 0::2, :], rq8[:, :, 0::2, :], tmp[:],
                        mybir.AluOpType.add)
                    nc.vector.tensor_tensor(
                        tmp[:], qb8[:, :, 0::2, :],
                        sin8[:, None, 1::2, :].to_broadcast([HD, HPC, 2, 32]),
                        mybir.AluOpType.mult)
                    nc.vector.tensor_tensor(
                        rq8[:, :, 1::2, :], rq8[:, :, 1::2, :], tmp[:],
                        mybir.AluOpType.add)
                    for hl in range(HPC):
                        ptr = pst.tile([HD, HD], MMD, tag="ptr",
                                       name=f"ptr_{name}_{tb}_{hl}")
                        nc.tensor.transpose(ptr[:], rq[:, hl * HD:(hl + 1) * HD],
                                            ident[:])
                        ob = op.tile([HD, HD], MMD, tag="ob",
                                     name=f"ob_{name}_{tb}_{hl}")
                        nc.scalar.copy(ob[:], ptr[:])
                        nc.sync.dma_start(outd[hl * HD:(hl + 1) * HD, tlsl],
                                          ob[:])

                pv = ps.tile([HD, EH], F32, tag="pqk", name=f"pv_{tb}")
                for d in range(NDC):
                    nc.tensor.matmul(pv[:], xo[:, d], wvS[:, d],
                                     start=(d == 0), stop=(d == NDC - 1))
                vb = op.tile([HD, EH], MMD, tag="vb", name=f"vb_{tb}")
                nc.vector.tensor_tensor(
                    vb[:], pv[:], bv_sb[:], mybir.AluOpType.add)
                for hl in range(HPC):
                    nc.sync.dma_start(v_db[b_][hl, :, tb % (NTB // B), :],
                                      vb[:, hl * HD:(hl + 1) * HD])
                if tb == NTB // B - 1:
                    # batch-0 spills complete: stream its first attention
                    # heads into SBUF while the batch-1 blocks project
                    load_qkv(0, 0)
                    load_qkv(0, 1)

        NPR = NKC // 2
        NQT = S // 512
        NES = D // 512
        with ExitStack() as ctx:
            pp = ctx.enter_context(tc.tile_pool(name="pp", bufs=5))
            accp = ctx.enter_context(tc.tile_pool(name="accp", bufs=2))
            ao = ctx.enter_context(tc.tile_pool(name="ao", bufs=4))
            wvf = ctx.enter_context(tc.tile_pool(name="wvf", bufs=1))
            wop = ctx.enter_context(tc.tile_pool(name="wop", bufs=6))
            oo = ctx.enter_context(tc.tile_pool(name="oo", bufs=4))
            psl = ctx.enter_context(tc.tile_pool(name="psl", bufs=3, space="PSUM"))
            pso = ctx.enter_context(tc.tile_pool(name="pso", bufs=2, space="PSUM"))

            bo_sb = wvf.tile([HD, D], F32)
            nc.sync.dma_start(bo_sb[:], bo[:])
            wvfS = {}
            wo_cache = {}

            def fire_a2a(b):
                nc.gpsimd.collective_compute(
                    "AllToAll", mybir.AluOpType.bypass,
                    replica_groups=[list(range(NC))],
                    ins=[cc_in[b][:]], outs=[cc_out[b][:]],
                )

            def stage_wvf(b):
                wvfS[b] = wvf.tile([HD, NDC, TPC], MMD, name=f"wvfS_{b}")
                t_ = wvfS[b]
                for i in range(NC):
                    nc.gpsimd.dma_start(
                        t_[:, i * HPC:(i + 1) * HPC, :],
                        cc_out[b][i].rearrange("(r1 p) c -> p r1 c", p=HD))

            def prefetch_wo(es):
                if es in wo_cache:
                    return
                wlo = wop.tile([HD, NDC // 2, 512], MMD, tag="wo")
                nc.sync.dma_start(wlo[:], woS[es, :, 0:NDC // 2])
                whi = wop.tile([HD, NDC // 2, 512], MMD, tag="wo")
                nc.scalar.dma_start(whi[:], woS[es, :, NDC // 2:NDC])
                wo_cache[es] = (wlo, whi)
                while len(wo_cache) > 3:
                    del wo_cache[next(iter(wo_cache))]

            def emit_d_unit(b, es, tb2):
                esl = slice(es * 512, (es + 1) * 512)
                prefetch_wo(es)
                wo_lo, wo_hi = wo_cache[es]
                pd = pso.tile([HD, 512], tag="po", dtype=F32,
                              name=f"pd_{b}_{es}_{tb2}")
                for di, d in enumerate(range(NDC)):
                    wo_t = wo_lo if d < NDC // 2 else wo_hi
                    nc.tensor.matmul(
                        pd[:],
                        wvfS[b][:, d, tb2 * HD:(tb2 + 1) * HD],
                        wo_t[:, d % (NDC // 2)],
                        start=(di == 0), stop=(di == NDC - 1))
                ob = oo.tile([HD, 512], F32, tag="obD",
                             name=f"obD_{b}_{es}_{tb2}")
                nc.vector.tensor_tensor(
                    ob[:], pd[:], bo_sb[:, esl], mybir.AluOpType.add)
                nc.scalar.dma_start(
                    out[b, tb2 * HD:(tb2 + 1) * HD, esl], ob[:])

            d_queue = []

            def drain_d(n=1):
                for _ in range(n):
                    if not d_queue:
                        return
                    b_, es_, tb2_ = d_queue.pop(0)
                    if d_queue:
                        prefetch_wo(d_queue[min(1, len(d_queue) - 1)][1])
                    emit_d_unit(b_, es_, tb2_)

            def emit_attention(b):
                for hl in range(HPC):
                    esl = slice(hl * HD, (hl + 1) * HD)
                    if (b, hl) not in qkv_tiles:
                        load_qkv(b, hl)
                    qh, kh, vh = qkv_tiles.pop((b, hl))

                    po = {}
                    acc = {}
                    pls = {}

                    def consume(qt, j):
                        pl = pls.pop((qt, j))
                        pe2 = pp.tile([HD, 1024], MMD, tag="pe",
                                      name=f"pe_{b}_{hl}_{qt}_{j}")
                        nc.scalar.activation(pe2[:], pl[:], AF.Exp,
                                             scale=SCALE, bias=expb_sb[:])
                        if j == 0:
                            nc.vector.tensor_tensor(
                                acc[qt][:], pe2[:, 0:512], pe2[:, 512:1024],
                                mybir.AluOpType.add)
                        else:
                            nc.vector.tensor_tensor(
                                acc[qt][:], acc[qt][:], pe2[:, 0:512],
                                mybir.AluOpType.add)
                            nc.vector.tensor_tensor(
                                acc[qt][:], acc[qt][:], pe2[:, 512:1024],
                                mybir.AluOpType.add)
                        nc.tensor.matmul(po[qt][:], vh[:, 2 * j],
                                         pe2[:, 0:512],
                                         start=(j == 0), stop=False)
                        nc.tensor.matmul(po[qt][:], vh[:, 2 * j + 1],
                                         pe2[:, 512:1024],
                                         start=False, stop=(j == NPR - 1))

                    def finish_qt(qt):
                        qsl = slice(qt * 512, (qt + 1) * 512)
                        su = psl.tile([HD, 1024], F32, tag="pl",
                                      name=f"su_{b}_{hl}_{qt}")
                        nc.tensor.matmul(su[:, 0:512], ones_sb[:], acc[qt][:],
                                         start=True, stop=True)
                        rec = ao.tile([HD, 512], F32, tag="rec",
                                      name=f"rec_{b}_{hl}_{qt}")
                        nc.vector.reciprocal_approx_fast(rec[:], su[:, 0:512])
                        osb = ao.tile([HD, 512], MMD, tag="osb",
                                      name=f"osb_{b}_{hl}_{qt}")
                        nc.vector.tensor_tensor(
                            osb[:], po[qt][:], rec[:], mybir.AluOpType.mult)
                        del po[qt], acc[qt]
                        for j2 in range(2):
                            j = qt * 2 + j2
                            nc.sync.dma_start(
                                cc_in[b][j, esl, :],
                                osb[:, j2 * TPC:(j2 + 1) * TPC])

                    units = [(qt, j) for qt in range(NQT) for j in range(NPR)]
                    for idx, (qt, j) in enumerate(units):
                        if j == 0:
                            po[qt] = pso.tile([HD, 512], tag="po", dtype=F32,
                                              name=f"po_{b}_{hl}_{qt}")
                            acc[qt] = accp.tile([HD, 512], MMD, tag="acc",
                                                name=f"acc_{b}_{hl}_{qt}")
                        qsl = slice(qt * 512, (qt + 1) * 512)
                        pl = psl.tile([HD, 1024], F32, tag="pl",
                                      name=f"pl_{b}_{hl}_{qt}_{j}")
                        nc.tensor.matmul(
                            pl[:, 0:512],
                            kh[:, (2 * j) * HD:(2 * j + 1) * HD], qh[:, qsl],
                            start=True, stop=True)
                        nc.tensor.matmul(
                            pl[:, 512:1024],
                            kh[:, (2 * j + 1) * HD:(2 * j + 2) * HD], qh[:, qsl],
                            start=True, stop=True)
                        pls[(qt, j)] = pl
                        if idx == NPR and not (b == B - 1 and hl == HPC - 1):
                            nb, nhl = (b, hl + 1) if hl + 1 < HPC else (b + 1, 0)
                            if (nb, nhl) not in qkv_tiles:
                                load_qkv(nb, nhl)
                        if idx >= 2:
                            consume(*units[idx - 2])
                        if j == 3 and qt > 0:
                            finish_qt(qt - 1)
                            if b == 1 and hl == 1 and qt == 1:
                                stage_wvf(0)
                                if d_queue:
                                    prefetch_wo(d_queue[0][1])
                            if (hl >= 3 or (hl == 2 and qt == 3)) \
                                    and d_queue:
                                drain_d(1)
                    consume(*units[-2])
                    consume(*units[-1])
                    finish_qt(NQT - 1)
                    if hl >= 2 and d_queue:
                        drain_d(2 if hl == 3 else 1)
                    if hl == 3:
                        fire_a2a(b)

            emit_attention(0)
            d_queue.extend((0, es, tb2)
                           for es in range(NES) for tb2 in range(TPC // HD))
            emit_attention(1)
            stage_wvf(1)
            d_queue.extend((1, es, tb2)
                           for es in reversed(range(NES))
                           for tb2 in range(TPC // HD))
            while d_queue:
                drain_d(1)

    nc.compile()
    return nc


def host_prep(x, position_ids, qkv_weight, qkv_bias, attn_out_weight,
              attn_out_bias):
    pos = np.asarray(position_ids).astype(np.int64)
    x = np.asarray(x, dtype=np.float32)
    Wqkv = np.asarray(qkv_weight, dtype=np.float32)
    bqkv = np.asarray(qkv_bias, dtype=np.float32)
    Wo = np.asarray(attn_out_weight, dtype=np.float32)
    bo = np.asarray(attn_out_bias, dtype=np.float32)

    xT = x.transpose(2, 1, 0).reshape(D, T)
    xS = _np_mmd(np.ascontiguousarray(
        xT.reshape(D // HD, HD, T // HD, HD).transpose(1, 2, 0, 3)))
    woS = _np_mmd(np.ascontiguousarray(
        Wo.T.reshape(D // HD, HD, D // 512, 512).transpose(2, 1, 0, 3)))

    cos_t, sin_t = _rope_tables()
    cosN = np.empty((T, HD), np.float32)
    sinN = np.empty((T, HD), np.float32)
    for b in range(B):
        rows = slice(b * S, (b + 1) * S)
        p1 = pos[b, 0, :]
        p2 = pos[b, 1, :]
        cosN[rows, 0:64] = cos_t[p1]
        cosN[rows, 64:128] = cos_t[p2]
        s1 = sin_t[p1].copy()
        s1[:, 0:32] *= -1.0
        s2 = sin_t[p2].copy()
        s2[:, 0:32] *= -1.0
        sinN[rows, 0:64] = s1
        sinN[rows, 64:128] = s2

    ones = _np_mmd(np.ones((HD, HD), np.float32))
    shared = dict(xS=xS, woS=woS, cosN=_np_mmd(cosN), sinN=_np_mmd(sinN),
                  onesc=ones,
                  bo=np.ascontiguousarray(np.broadcast_to(bo, (HD, D))))

    in_maps = []
    for c in range(NC):
        heads = range(HPC * c, HPC * (c + 1))
        wq = np.concatenate([Wqkv[384 * h: 384 * h + 128] for h in heads])
        wk = np.concatenate([Wqkv[384 * h + 128: 384 * h + 256] for h in heads])
        wv = np.concatenate([Wqkv[384 * h + 256: 384 * h + 384] for h in heads])
        in_maps.append(dict(
            shared,
            wqT=_np_mmd(np.ascontiguousarray(
                wq.T.reshape(D // HD, HD, EH).transpose(1, 0, 2))),
            wkT=_np_mmd(np.ascontiguousarray(
                wk.T.reshape(D // HD, HD, EH).transpose(1, 0, 2))),
            wvT=_np_mmd(np.ascontiguousarray(
                wv.T.reshape(D // HD, HD, EH).transpose(1, 0, 2))),
            bq=np.ascontiguousarray(np.broadcast_to(np.concatenate(
                [bqkv[384 * h: 384 * h + 128] for h in heads]), (HD, EH))),
            bk=np.ascontiguousarray(np.broadcast_to(np.concatenate(
                [bqkv[384 * h + 128: 384 * h + 256] for h in heads]), (HD, EH))),
            bv=np.ascontiguousarray(np.broadcast_to(np.concatenate(
                [bqkv[384 * h + 256: 384 * h + 384] for h in heads]), (HD, EH))),
        ))
    return in_maps


def kernel(x, position_ids, qkv_weight, qkv_bias, attn_out_weight,
           attn_out_bias, _trace=False):
    if "nc" not in _cache:
        _cache["nc"] = build_program()
    nc = _cache["nc"]

    in_maps = host_prep(x, position_ids, qkv_weight, qkv_bias,
                        attn_out_weight, attn_out_bias)
    res = run_bass_kernel_spmd(nc, in_maps, core_ids=list(range(NC)),
                               trace=_trace)
    _cache["last_result"] = res

    out = np.empty((S, B, D), np.float32)
    for c in range(NC):
        oc = res.results[c]["out"]
        for b in range(B):
            out[TPC * c: TPC * (c + 1), b, :] = oc[b]
    return out


# revision 33
# speedup vs baseline: 1.0075x; 1.0009x over previous
"""Trainium2 Bass kernel for nn_Attention_GLM_Wrapped (S=2048, B=2, D=4096, H=32).

v2 fallback: baseline structure + startup/queue/prefetch scheduling fixes.
Measured 1,460,426 ns, rel err 1.395e-3.
"""
import sys

sys.path.insert(0, "/opt/trn_rl_repo")

import numpy as np
import ml_dtypes
from contextlib import ExitStack

import concourse.bass as bass
from concourse import bacc
import concourse.mybir as mybir
import concourse.tile as tile
from concourse.bass_utils import run_bass_kernel_spmd
from concourse.masks import make_identity

F32 = mybir.dt.float32
BF16 = mybir.dt.bfloat16
FP16 = mybir.dt.float16
AF = mybir.ActivationFunctionType

MMD = FP16
EXPB = -10.0

S, B, D = 2048, 2, 4096
H = 32
HD = 128
T = S * B
NC = 8
HPC = H // NC
EH = HPC * HD
TPC = T // NC // B
SCALE = float(1.0 / np.sqrt(HD))

_cache = {}


def _np_mmd(a):
    if MMD == BF16:
        return np.asarray(a, np.float32).astype(ml_dtypes.bfloat16)
    if MMD == FP16:
        return np.asarray(a, np.float32).astype(np.float16)
    return np.ascontiguousarray(np.asarray(a, np.float32))


def _rope_tables():
    rot = 64
    inv_freq = 1.0 / (10000.0 ** (np.arange(0, rot, 2, dtype=np.float32) / rot))
    v = np.arange(S, dtype=np.float32)[:, None] * inv_freq[None, :]
    v = np.concatenate([v, v], axis=-1)
    return np.cos(v).astype(np.float32), np.sin(v).astype(np.float32)


def build_program():
    nc = bacc.Bacc("TRN2", target_bir_lowering=False, debug=False, num_devices=NC)

    xS = nc.dram_tensor("xS", [HD, T // HD, D // HD, HD], MMD,
                        kind="ExternalInput").ap()
    wqT = nc.dram_tensor("wqT", [HD, D // HD, EH], MMD, kind="ExternalInput").ap()
    wkT = nc.dram_tensor("wkT", [HD, D // HD, EH], MMD, kind="ExternalInput").ap()
    wvT = nc.dram_tensor("wvT", [HD, D // HD, EH], MMD, kind="ExternalInput").ap()
    woS = nc.dram_tensor("woS", [D // 512, HD, D // HD, 512], MMD,
                         kind="ExternalInput").ap()
    bq = nc.dram_tensor("bq", [HD, EH], F32, kind="ExternalInput").ap()
    bk = nc.dram_tensor("bk", [HD, EH], F32, kind="ExternalInput").ap()
    bv = nc.dram_tensor("bv", [HD, EH], F32, kind="ExternalInput").ap()
    bo = nc.dram_tensor("bo", [HD, D], F32, kind="ExternalInput").ap()
    cosN = nc.dram_tensor("cosN", [T, HD], MMD, kind="ExternalInput").ap()
    sinN = nc.dram_tensor("sinN", [T, HD], MMD, kind="ExternalInput").ap()
    onesc = nc.dram_tensor("onesc", [HD, HD], MMD, kind="ExternalInput").ap()
    out = nc.dram_tensor("out", [B, TPC, D], F32, kind="ExternalOutput").ap()

    NTB = T // HD
    NDC = D // HD

    with tile.TileContext(nc) as tc, ExitStack() as top:
        dram = top.enter_context(tc.tile_pool(name="dram", bufs=1, space="DRAM"))
        cpool = top.enter_context(tc.tile_pool(name="cpool", bufs=1))

        # per-batch spill tiles: batch-0 attention loads depend only on the
        # batch-0 half of phase A, so its q/k/v can stream in early
        qT_db = [dram.tile([EH, S], MMD, name=f"qT_d{b}") for b in range(B)]
        kT_db = [dram.tile([EH, S], MMD, name=f"kT_d{b}") for b in range(B)]
        v_db = [dram.tile([HPC, HD, S // HD, HD], MMD, name=f"v_d{b}")
                for b in range(B)]
        cc_in = [dram.tile([NC, EH, TPC], MMD, name=f"cc_in_{b}")
                 for b in range(B)]
        cc_out = [dram.tile([NC, EH, TPC], MMD, name=f"cc_out_{b}")
                  for b in range(B)]

        ident = cpool.tile([HD, HD], MMD)
        make_identity(nc, ident)

        with ExitStack() as wctx:
            pw = wctx.enter_context(tc.tile_pool(name="pw", bufs=1, space="PSUM"))
            pwr = cpool.tile([HD, 512], MMD)
            nc.vector.memset(pwr[:], 0.0)
            pwt = pw.tile([HD, 512], F32)
            NPW = 150
            for i in range(NPW):
                nc.tensor.matmul(pwt[:], ident[:], pwr[:],
                                 start=(i == 0), stop=(i == NPW - 1))

        ones_sb = cpool.tile([HD, HD], MMD)
        nc.gpsimd.dma_start(ones_sb[:], onesc[:])
        bq_sb = cpool.tile([HD, EH], F32)
        nc.gpsimd.dma_start(bq_sb[:], bq[:])
        bk_sb = cpool.tile([HD, EH], F32)
        nc.gpsimd.dma_start(bk_sb[:], bk[:])
        bv_sb = cpool.tile([HD, EH], F32)
        nc.gpsimd.dma_start(bv_sb[:], bv[:])
        expb_sb = cpool.tile([HD, 1], F32)
        nc.vector.memset(expb_sb[:], EXPB)

        # attention input pool lives at top level so batch-0 head loads can
        # be issued from inside phase A (overlapping the batch-1 blocks)
        NKC = S // HD
        qk = top.enter_context(tc.tile_pool(name="qk", bufs=2))
        qkv_tiles = {}

        def load_qkv(b, hl):
            esl = slice(hl * HD, (hl + 1) * HD)
            qh = qk.tile([HD, S], MMD, tag="qh", name=f"qh_{b}_{hl}")
            nc.gpsimd.dma_start(qh[:], qT_db[b][esl, :])
            kh = qk.tile([HD, S], MMD, tag="kh", name=f"kh_{b}_{hl}")
            nc.gpsimd.dma_start(kh[:], kT_db[b][esl, :])
            vh = qk.tile([HD, NKC, HD], MMD, tag="vh", name=f"vh_{b}_{hl}")
            nc.gpsimd.dma_start(vh[:], v_db[b][hl])
            qkv_tiles[(b, hl)] = (qh, kh, vh)

        with ExitStack() as ctx:
            wres = ctx.enter_context(tc.tile_pool(name="wres", bufs=1))
            xp = ctx.enter_context(tc.tile_pool(name="xp", bufs=3))
            rp = ctx.enter_context(tc.tile_pool(name="rp", bufs=3))
            op = ctx.enter_context(tc.tile_pool(name="op", bufs=6))
            ps = ctx.enter_context(tc.tile_pool(name="psA", bufs=4, space="PSUM"))
            pst = ctx.enter_context(tc.tile_pool(name="psAt", bufs=4, space="PSUM"))

            wqS = wres.tile([HD, NDC, EH], MMD)
            wkS = wres.tile([HD, NDC, EH], MMD)
            wvS = wres.tile([HD, NDC, EH], MMD)
            xo0 = xp.tile([HD, NDC, HD], MMD, tag="xo", name="xo_0")
            nc.scalar.dma_start(xo0[:], xS[:, 0])
            cos0 = xp.tile([HD, HD], MMD, tag="cos", name="cos_0")
            nc.gpsimd.dma_start(cos0[:], cosN[0:HD, :])
            sin0 = xp.tile([HD, HD], MMD, tag="sin", name="sin_0")
            nc.gpsimd.dma_start(sin0[:], sinN[0:HD, :])
            for ch in range(8):
                csl = slice(ch * NDC // 8, (ch + 1) * NDC // 8)
                for wS, wsrc, q in ((wqS, wqT, nc.sync), (wkS, wkT, nc.scalar),
                                    (wvS, wvT, nc.gpsimd)):
                    q.dma_start(wS[:, csl], wsrc[:, csl])

            for tb in range(NTB):
                tsl = slice(tb * HD, (tb + 1) * HD)
                if tb == 0:
                    xo, cos_t, sin_t = xo0, cos0, sin0
                else:
                    xo = xp.tile([HD, NDC, HD], MMD, tag="xo")
                    nc.scalar.dma_start(xo[:], xS[:, tb])
                    cos_t = xp.tile([HD, HD], MMD, tag="cos")
                    nc.scalar.dma_start(cos_t[:], cosN[tsl, :])
                    sin_t = xp.tile([HD, HD], MMD, tag="sin")
                    nc.scalar.dma_start(sin_t[:], sinN[tsl, :])

                b_ = tb // (NTB // B)
                tlsl = slice((tb % (NTB // B)) * HD,
                             (tb % (NTB // B) + 1) * HD)
                for name, wS, b_sb in (("q", wqS, bq_sb), ("k", wkS, bk_sb)):
                    outd = qT_db[b_] if name == "q" else kT_db[b_]
                    pq = ps.tile([HD, EH], F32, tag="pqk", name=f"pqk_{name}_{tb}")
                    for d in range(NDC):
                        nc.tensor.matmul(pq[:], xo[:, d], wS[:, d],
                                         start=(d == 0), stop=(d == NDC - 1))
                    qb = rp.tile([HD, EH], MMD, tag="qb", name=f"qb_{name}_{tb}")
                    nc.vector.tensor_tensor(
                        qb[:], pq[:], b_sb[:], mybir.AluOpType.add)
                    rq = rp.tile([HD, EH], MMD, tag="rq", name=f"rq_{name}_{tb}")
                    qb4 = qb.rearrange("p (h e) -> p h e", h=HPC)
                    rq4 = rq.rearrange("p (h e) -> p h e", h=HPC)
                    cosb = cos_t[:, None, :].to_broadcast([HD, HPC, HD])
                    nc.vector.tensor_tensor(rq4[:], qb4[:], cosb,
                                            mybir.AluOpType.mult)
                    qb8 = qb.rearrange("p (h u e) -> p h u e", h=HPC, u=4)
                    rq8 = rq.rearrange("p (h u e) -> p h u e", h=HPC, u=4)
                    sin8 = sin_t.rearrange("p (u e) -> p u e", u=4)
                    tmp = rp.tile([HD, HPC, 2, 32], MMD, tag="tmp",
                                  name=f"tmp_{name}_{tb}")
                    nc.vector.tensor_tensor(
                        tmp[:], qb8[:, :, 1::2, :],
                        sin8[:, None, 0::2, :].to_broadcast([HD, HPC, 2, 32]),
                        mybir.AluOpType.mult)
                    nc.vector.tensor_tensor(
                        rq8[:, :, 0::2, :], rq8[:, :, 0::2, :], tmp[:],
                        mybir.AluOpType.add)
                    nc.vector.tensor_tensor(
                        tmp[:], qb8[:, :, 0::2, :],
                        sin8[:, None, 1::2, :].to_broadcast([HD, HPC, 2, 32]),
                        mybir.AluOpType.mult)
                    nc.vector.tensor_tensor(
                        rq8[:, :, 1::2, :], rq8[:, :, 1::2, :], tmp[:],
                        mybir.AluOpType.add)
                    for hl in range(HPC):
                        ptr = pst.tile([HD, HD], MMD, tag="ptr",
                                       name=f"ptr_{name}_{tb}_{hl}")
                        nc.tensor.transpose(ptr[:], rq[:, hl * HD:(hl + 1) * HD],
                                            ident[:])
                        ob = op.tile([HD, HD], MMD, tag="ob",
                                     name=f"ob_{name}_{tb}_{hl}")
                        nc.scalar.copy(ob[:], ptr[:])
                        nc.sync.dma_start(outd[hl * HD:(hl + 1) * HD, tlsl],
                                          ob[:])

                pv = ps.tile([HD, EH], F32, tag="pqk", name=f"pv_{tb}")
                for d in range(NDC):
                    nc.tensor.matmul(pv[:], xo[:, d], wvS[:, d],
                                     start=(d == 0), stop=(d == NDC - 1))
                vb = op.tile([HD, EH], MMD, tag="vb", name=f"vb_{tb}")
                nc.vector.tensor_tensor(
                    vb[:], pv[:], bv_sb[:], mybir.AluOpType.add)
                for hl in range(HPC):
                    nc.sync.dma_start(v_db[b_][hl, :, tb % (NTB // B), :],
                                      vb[:, hl * HD:(hl + 1) * HD])
                if tb == NTB // B - 1:
                    # batch-0 spills complete: stream its first attention
                    # heads into SBUF while the batch-1 blocks project
                    load_qkv(0, 0)
                    load_qkv(0, 1)

        NPR = NKC // 2
        NQT = S // 512
        NES = D // 512
        with ExitStack() as ctx:
            pp = ctx.enter_context(tc.tile_pool(name="pp", bufs=5))
            accp = ctx.enter_context(tc.tile_pool(name="accp", bufs=2))
            ao = ctx.enter_context(tc.tile_pool(name="ao", bufs=4))
            wvf = ctx.enter_context(tc.tile_pool(name="wvf", bufs=1))
            wop = ctx.enter_context(tc.tile_pool(name="wop", bufs=6))
            oo = ctx.enter_context(tc.tile_pool(name="oo", bufs=4))
            psl = ctx.enter_context(tc.tile_pool(name="psl", bufs=3, space="PSUM"))
            pso = ctx.enter_context(tc.tile_pool(name="pso", bufs=2, space="PSUM"))

            bo_sb = wvf.tile([HD, D], F32)
            nc.sync.dma_start(bo_sb[:], bo[:])
            wvfS = {}
            wo_cache = {}

            def fire_a2a(b):
                nc.gpsimd.collective_compute(
                    "AllToAll", mybir.AluOpType.bypass,
                    replica_groups=[list(range(NC))],
                    ins=[cc_in[b][:]], outs=[cc_out[b][:]],
                )

            def stage_wvf(b):
                wvfS[b] = wvf.tile([HD, NDC, TPC], MMD, name=f"wvfS_{b}")
                t_ = wvfS[b]
                for i in range(NC):
                    nc.gpsimd.dma_start(
                        t_[:, i * HPC:(i + 1) * HPC, :],
                        cc_out[b][i].rearrange("(r1 p) c -> p r1 c", p=HD))

            def prefetch_wo(es):
                if es in wo_cache:
                    return
                wlo = wop.tile([HD, NDC // 2, 512], MMD, tag="wo")
                nc.sync.dma_start(wlo[:], woS[es, :, 0:NDC // 2])
                whi = wop.tile([HD, NDC // 2, 512], MMD, tag="wo")
                nc.scalar.dma_start(whi[:], woS[es, :, NDC // 2:NDC])
                wo_cache[es] = (wlo, whi)
                while len(wo_cache) > 3:
                    del wo_cache[next(iter(wo_cache))]

            def emit_d_unit(b, es, tb2):
                esl = slice(es * 512, (es + 1) * 512)
                prefetch_wo(es)
                wo_lo, wo_hi = wo_cache[es]
                pd = pso.tile([HD, 512], tag="po", dtype=F32,
                              name=f"pd_{b}_{es}_{tb2}")
                for di, d in enumerate(range(NDC)):
                    wo_t = wo_lo if d < NDC // 2 else wo_hi
                    nc.tensor.matmul(
                        pd[:],
                        wvfS[b][:, d, tb2 * HD:(tb2 + 1) * HD],
                        wo_t[:, d % (NDC // 2)],
                        start=(di == 0), stop=(di == NDC - 1))
                ob = oo.tile([HD, 512], F32, tag="obD",
                             name=f"obD_{b}_{es}_{tb2}")
                nc.vector.tensor_tensor(
                    ob[:], pd[:], bo_sb[:, esl], mybir.AluOpType.add)
                nc.scalar.dma_start(
                    out[b, tb2 * HD:(tb2 + 1) * HD, esl], ob[:])

            d_queue = []

            def drain_d(n=1):
                for _ in range(n):
                    if not d_queue:
                        return
                    b_, es_, tb2_ = d_queue.pop(0)
                    if d_queue:
                        prefetch_wo(d_queue[min(1, len(d_queue) - 1)][1])
                    emit_d_unit(b_, es_, tb2_)

            def emit_attention(b):
                for hl in range(HPC):
                    esl = slice(hl * HD, (hl + 1) * HD)
                    if (b, hl) not in qkv_tiles:
                        load_qkv(b, hl)
                    qh, kh, vh = qkv_tiles.pop((b, hl))

                    po = {}
                    acc = {}
                    pls = {}

                    def consume(qt, j):
                        pl = pls.pop((qt, j))
                        pe2 = pp.tile([HD, 1024], MMD, tag="pe",
                                      name=f"pe_{b}_{hl}_{qt}_{j}")
                        nc.scalar.activation(pe2[:], pl[:], AF.Exp,
                                             scale=SCALE, bias=expb_sb[:])
                        if j == 0:
                            nc.vector.tensor_tensor(
                                acc[qt][:], pe2[:, 0:512], pe2[:, 512:1024],
                                mybir.AluOpType.add)
                        else:
                            nc.vector.tensor_tensor(
                                acc[qt][:], acc[qt][:], pe2[:, 0:512],
                                mybir.AluOpType.add)
                            nc.vector.tensor_tensor(
                                acc[qt][:], acc[qt][:], pe2[:, 512:1024],
                                mybir.AluOpType.add)
                        nc.tensor.matmul(po[qt][:], vh[:, 2 * j],
                                         pe2[:, 0:512],
                                         start=(j == 0), stop=False)
                        nc.tensor.matmul(po[qt][:], vh[:, 2 * j + 1],
                                         pe2[:, 512:1024],
                                         start=False, stop=(j == NPR - 1))

                    def finish_qt(qt):
                        qsl = slice(qt * 512, (qt + 1) * 512)
                        su = psl.tile([HD, 1024], F32, tag="pl",
                                      name=f"su_{b}_{hl}_{qt}")
                        nc.tensor.matmul(su[:, 0:512], ones_sb[:], acc[qt][:],
                                         start=True, stop=True)
                        rec = ao.tile([HD, 512], F32, tag="rec",
                                      name=f"rec_{b}_{hl}_{qt}")
                        nc.vector.reciprocal_approx_fast(rec[:], su[:, 0:512])
                        osb = ao.tile([HD, 512], MMD, tag="osb",
                                      name=f"osb_{b}_{hl}_{qt}")
                        nc.vector.tensor_tensor(
                            osb[:], po[qt][:], rec[:], mybir.AluOpType.mult)
                        del po[qt], acc[qt]
                        for j2 in range(2):
                            j = qt * 2 + j2
                            nc.sync.dma_start(
                                cc_in[b][j, esl, :],
                                osb[:, j2 * TPC:(j2 + 1) * TPC])

                    units = [(qt, j) for qt in range(NQT) for j in range(NPR)]
                    for idx, (qt, j) in enumerate(units):
                        if j == 0:
                            po[qt] = pso.tile([HD, 512], tag="po", dtype=F32,
                                              name=f"po_{b}_{hl}_{qt}")
                            acc[qt] = accp.tile([HD, 512], MMD, tag="acc",
                                                name=f"acc_{b}_{hl}_{qt}")
                        qsl = slice(qt * 512, (qt + 1) * 512)
                        pl = psl.tile([HD, 1024], F32, tag="pl",
                                      name=f"pl_{b}_{hl}_{qt}_{j}")
                        nc.tensor.matmul(
                            pl[:, 0:512],
                            kh[:, (2 * j) * HD:(2 * j + 1) * HD], qh[:, qsl],
                            start=True, stop=True)
                        nc.tensor.matmul(
                            pl[:, 512:1024],
                            kh[:, (2 * j + 1) * HD:(2 * j + 2) * HD], qh[:, qsl],
                            start=True, stop=True)
                        pls[(qt, j)] = pl
                        if idx == NPR and not (b == B - 1 and hl == HPC - 1):
                            nb, nhl = (b, hl + 1) if hl + 1 < HPC else (b + 1, 0)
                            if (nb, nhl) not in qkv_tiles:
                                load_qkv(nb, nhl)
                        if idx >= 2:
                            consume(*units[idx - 2])
                        if j == 3 and qt > 0:
                            finish_qt(qt - 1)
                            if b == 1 and hl == 1 and qt == 1:
                                stage_wvf(0)
                                if d_queue:
                                    prefetch_wo(d_queue[0][1])
                            if hl >= 3 and d_queue:
                                drain_d(1)
                    consume(*units[-2])
                    consume(*units[-1])
                    finish_qt(NQT - 1)
                    if hl >= 2 and d_queue:
                        drain_d(4 if hl == 3 else 1)
                    if hl == 3:
                        fire_a2a(b)

            emit_attention(0)
            d_queue.extend((0, es, tb2)
                           for es in range(NES) for tb2 in range(TPC // HD))
            emit_attention(1)
            stage_wvf(1)
            d_queue.extend((1, es, tb2)
                           for es in reversed(range(NES))
                           for tb2 in range(TPC // HD))
            while d_queue:
                drain_d(1)

    nc.compile()
    return nc


def host_prep(x, position_ids, qkv_weight, qkv_bias, attn_out_weight,
              attn_out_bias):
    pos = np.asarray(position_ids).astype(np.int64)
    x = np.asarray(x, dtype=np.float32)
    Wqkv = np.asarray(qkv_weight, dtype=np.float32)
    bqkv = np.asarray(qkv_bias, dtype=np.float32)
    Wo = np.asarray(attn_out_weight, dtype=np.float32)
    bo = np.asarray(attn_out_bias, dtype=np.float32)

    xT = x.transpose(2, 1, 0).reshape(D, T)
    xS = _np_mmd(np.ascontiguousarray(
        xT.reshape(D // HD, HD, T // HD, HD).transpose(1, 2, 0, 3)))
    woS = _np_mmd(np.ascontiguousarray(
        Wo.T.reshape(D // HD, HD, D // 512, 512).transpose(2, 1, 0, 3)))

    cos_t, sin_t = _rope_tables()
    cosN = np.empty((T, HD), np.float32)
    sinN = np.empty((T, HD), np.float32)
    for b in range(B):
        rows = slice(b * S, (b + 1) * S)
        p1 = pos[b, 0, :]
        p2 = pos[b, 1, :]
        cosN[rows, 0:64] = cos_t[p1]
        cosN[rows, 64:128] = cos_t[p2]
        s1 = sin_t[p1].copy()
        s1[:, 0:32] *= -1.0
        s2 = sin_t[p2].copy()
        s2[:, 0:32] *= -1.0
        sinN[rows, 0:64] = s1
        sinN[rows, 64:128] = s2

    ones = _np_mmd(np.ones((HD, HD), np.float32))
    shared = dict(xS=xS, woS=woS, cosN=_np_mmd(cosN), sinN=_np_mmd(sinN),
                  onesc=ones,
                  bo=np.ascontiguousarray(np.broadcast_to(bo, (HD, D))))

    in_maps = []
    for c in range(NC):
        heads = range(HPC * c, HPC * (c + 1))
        wq = np.concatenate([Wqkv[384 * h: 384 * h + 128] for h in heads])
        wk = np.concatenate([Wqkv[384 * h + 128: 384 * h + 256] for h in heads])
        wv = np.concatenate([Wqkv[384 * h + 256: 384 * h + 384] for h in heads])
        in_maps.append(dict(
            shared,
            wqT=_np_mmd(np.ascontiguousarray(
                wq.T.reshape(D // HD, HD, EH).transpose(1, 0, 2))),
            wkT=_np_mmd(np.ascontiguousarray(
                wk.T.reshape(D // HD, HD, EH).transpose(1, 0, 2))),
            wvT=_np_mmd(np.ascontiguousarray(
                wv.T.reshape(D // HD, HD, EH).transpose(1, 0, 2))),
            bq=np.ascontiguousarray(np.broadcast_to(np.concatenate(
                [bqkv[384 * h: 384 * h + 128] for h in heads]), (HD, EH))),
            bk=np.ascontiguousarray(np.broadcast_to(np.concatenate(
                [bqkv[384 * h + 128: 384 * h + 256] for h in heads]), (HD, EH))),
            bv=np.ascontiguousarray(np.broadcast_to(np.concatenate(
                [bqkv[384 * h + 256: 384 * h + 384] for h in heads]), (HD, EH))),
        ))
    return in_maps


def kernel(x, position_ids, qkv_weight, qkv_bias, attn_out_weight,
           attn_out_bias, _trace=False):
    if "nc" not in _cache:
        _cache["nc"] = build_program()
    nc = _cache["nc"]

    in_maps = host_prep(x, position_ids, qkv_weight, qkv_bias,
                        attn_out_weight, attn_out_bias)
    res = run_bass_kernel_spmd(nc, in_maps, core_ids=list(range(NC)),
                               trace=_trace)
    _cache["last_result"] = res

    out = np.empty((S, B, D), np.float32)
    for c in range(NC):
        oc = res.results[c]["out"]
        for b in range(B):
            out[TPC * c: TPC * (c + 1), b, :] = oc[b]
    return out
